# revision 9
# baseline (speedup 1.0000x reference)
"""Trainium2 Bass kernel for EnhancedRelativeGlobalAttention (B=8, L=1024, D=512, H=8).

Strategy: data-parallel over batch (1 batch element per NeuronCore, 8 cores).
Per core:
  - QKV projections with PE (f32r matmuls), Q/K in transposed layout [D, L],
    V in natural layout [L, D].
  - Per head: rel scores R = q_h @ rel_emb^T computed triangularly, written to
    a flat DRAM scratch (bf16); the Music-Transformer "skew" becomes a strided
    DMA read of that flat buffer (offset 1023, row-step 1023).
  - Logits accumulated in PSUM: scores matmul + skew added via identity-matmul
    + causal mask added via identity-matmul of a -24000 triangular block.
    exp on ScalarE with fused per-row accumulation (softmax denominators).
  - Normalize on VectorE (per-partition reciprocal scalar) -> attn output,
    PE-transpose of attn blocks feeds the PV matmul; O-projection at the end.
  - Causal structure: strictly-upper 128x128 blocks are never computed; the
    output buffer is pre-zeroed by the runtime.
"""
import sys
import os

sys.path.insert(0, '/opt/trn_rl_repo')

import numpy as np
import ml_dtypes

import concourse.bass as bass
import concourse.mybir as mybir
import concourse.tile as tile
from concourse import bacc
from concourse.bass_utils import run_bass_kernel_spmd

F32 = mybir.dt.float32
BF16 = mybir.dt.bfloat16
F32R = mybir.dt.float32r

B, L, D, H = 8, 1024, 512, 8
HD = D // H          # 64
NB = L // 128        # 8 l-blocks
KC = D // 128        # 4 contraction chunks

_cache = {}


def _nsplit(W):
    """Split [0, W) into PSUM-bank-aligned matmul chunks (<=512 each)."""
    if W <= 512:
        return [(0, W)]
    return [(0, 512), (512, W - 512)]


def build_program():
    nc = bacc.Bacc("TRN2", target_bir_lowering=False, debug=False, num_devices=8)

    # ---- DRAM I/O ----
    xqt = nc.dram_tensor("xqt", [128, KC, L], F32R, kind="ExternalInput")
    xkt = nc.dram_tensor("xkt", [128, KC, L], F32R, kind="ExternalInput")
    xvt = nc.dram_tensor("xvt", [128, KC, L], F32R, kind="ExternalInput")
    wq = nc.dram_tensor("wq", [128, KC, D], F32R, kind="ExternalInput")
    wk = nc.dram_tensor("wk", [128, KC, D], F32R, kind="ExternalInput")
    wv = nc.dram_tensor("wv", [128, KC, D], F32R, kind="ExternalInput")
    wo = nc.dram_tensor("wo", [128, KC, D], F32R, kind="ExternalInput")
    bq2 = nc.dram_tensor("bq2", [128, KC], F32, kind="ExternalInput")
    bk2 = nc.dram_tensor("bk2", [128, KC], F32, kind="ExternalInput")
    bv1 = nc.dram_tensor("bv1", [D], F32, kind="ExternalInput")
    bo1 = nc.dram_tensor("bo1", [D], F32, kind="ExternalInput")
    ft2 = nc.dram_tensor("ft2", [128, L], F32R, kind="ExternalInput")   # rel_emb^T stacked twice
    identbf = nc.dram_tensor("identbf", [128, 128], BF16, kind="ExternalInput")
    negtriubf = nc.dram_tensor("negtriubf", [128, 128], BF16, kind="ExternalInput")
    identf = nc.dram_tensor("identf", [128, 128], F32, kind="ExternalInput")
    zerobf = nc.dram_tensor("zerobf", [128, 128], BF16, kind="ExternalInput")

    out_d = nc.dram_tensor("out", [L, D], F32, kind="ExternalOutput")
    attn_d = nc.dram_tensor("attn", [H, L, L], F32, kind="ExternalOutput")
    rbufs = [nc.dram_tensor(f"r{h}", [L, L], BF16) for h in range(H)]

    with tile.TileContext(nc) as tc:
        with (
            tc.tile_pool(name="consts", bufs=1) as consts,
            tc.tile_pool(name="persist", bufs=1) as persist,
            tc.tile_pool(name="psA", bufs=2, space="PSUM") as psA,
            tc.tile_pool(name="psT", bufs=2, space="PSUM") as psT,
            tc.tile_pool(name="psO", bufs=2, space="PSUM") as psO,
            tc.tile_pool(name="rwork", bufs=2) as rwork,
            tc.tile_pool(name="swork", bufs=2) as swork,
            tc.tile_pool(name="pwork", bufs=2) as pwork,
            tc.tile_pool(name="awork", bufs=2) as awork,
            tc.tile_pool(name="tiny", bufs=4) as tiny,
        ):
            # ---- constants ----
            ft_sb = consts.tile([128, L], F32R, tag="ft")
            nc.sync.dma_start(out=ft_sb, in_=ft2[:])
            ident_bf = consts.tile([128, 128], BF16, tag="identbf")
            nc.sync.dma_start(out=ident_bf, in_=identbf[:])
            negtriu_bf = consts.tile([128, 128], BF16, tag="negtriu")
            nc.sync.dma_start(out=negtriu_bf, in_=negtriubf[:])
            ident_f = consts.tile([128, 128], F32, tag="identf")
            nc.sync.dma_start(out=ident_f, in_=identf[:])
            zero_bf = consts.tile([128, 128], BF16, tag="zerobf")
            nc.sync.dma_start(out=zero_bf, in_=zerobf[:])
            bq_sb = consts.tile([128, KC], F32, tag="bq")
            nc.sync.dma_start(out=bq_sb, in_=bq2[:])
            bk_sb = consts.tile([128, KC], F32, tag="bk")
            nc.sync.dma_start(out=bk_sb, in_=bk2[:])
            bvb = consts.tile([128, D], F32, tag="bvb")
            nc.sync.dma_start(out=bvb, in_=bass.AP(tensor=bv1, offset=0, ap=[[0, 128], [1, D]]))
            bob = consts.tile([128, D], F32, tag="bob")
            nc.sync.dma_start(out=bob, in_=bass.AP(tensor=bo1, offset=0, ap=[[0, 128], [1, D]]))

            # ---- persistent activations ----
            qt_sb = persist.tile([128, KC, L], F32R, tag="qt")      # q^T  [D, L]
            kt_sb = persist.tile([128, KC, L], F32R, tag="kt")      # k^T  [D, L]
            v_sb = persist.tile([128, NB, D], F32R, tag="v")        # v natural [L, D]
            wo_sb = persist.tile([128, KC, D], F32R, tag="wo")
            outT_sb = persist.tile([128, KC, L], F32R, tag="outT")  # attn_out^T [D, L]
            attnT_sb = persist.tile([128, NB, L], F32R, tag="attnT")  # ring: [j, jb, 2x512 l-cols]
            nc.sync.dma_start(out=wo_sb, in_=wo[:])

            # ---- projections ----
            with (
                tc.tile_pool(name="xbuf", bufs=2) as xbuf,
                tc.tile_pool(name="wstream", bufs=8) as wstream,
                tc.tile_pool(name="wvres", bufs=1) as wvres,
            ):
                # qT / kT: out[dchunk][128, L] = sum_kc W[kc][:, dcols].T @ xT[kc]
                for (xd, wd, dst, b_sb) in (
                    (xqt, wq, qt_sb, bq_sb),
                    (xkt, wk, kt_sb, bk_sb),
                ):
                    x_sb = xbuf.tile([128, KC, L], F32R, tag="x")
                    nc.sync.dma_start(out=x_sb, in_=xd[:])
                    for c in range(KC):
                        ps = psA.tile([128, L], F32, tag="big")
                        for kc in range(KC):
                            wch = wstream.tile([128, 128], F32R, tag="wch")
                            nc.sync.dma_start(
                                out=wch, in_=wd[:, kc, 128 * c:128 * c + 128])
                            for (ns, wid) in _nsplit(L):
                                nc.tensor.matmul(
                                    ps[:, ns:ns + wid],
                                    wch,
                                    x_sb[:, kc, ns:ns + wid],
                                    start=(kc == 0),
                                    stop=(kc == KC - 1),
                                )
                        nc.scalar.activation(
                            dst[:, c, :], ps,
                            mybir.ActivationFunctionType.Identity,
                            bias=b_sb[:, c:c + 1], scale=1.0,
                        )
                # v natural: v[jb][128, D] = sum_kc xvT[kc][:, jcols].T @ Wv[kc]
                xv_sb = xbuf.tile([128, KC, L], F32R, tag="x")
                nc.sync.dma_start(out=xv_sb, in_=xvt[:])
                wv_sb = wvres.tile([128, KC, D], F32R, tag="wv")
                nc.sync.dma_start(out=wv_sb, in_=wv[:])
                for jb in range(NB):
                    ps = psA.tile([128, D], F32, tag="big")
                    for kc in range(KC):
                        nc.tensor.matmul(
                            ps,
                            xv_sb[:, kc, 128 * jb:128 * jb + 128],
                            wv_sb[:, kc, :],
                            start=(kc == 0),
                            stop=(kc == KC - 1),
                        )
                    nc.vector.scalar_tensor_tensor(
                        v_sb[:, jb, :], ps, 1.0, bvb,
                        op0=mybir.AluOpType.mult, op1=mybir.AluOpType.add,
                    )

            def qh(h, lb):
                """lhsT slice of q^T for head h, l-block lb: [64, 128]."""
                p0 = 64 * (h % 2)
                return qt_sb[p0:p0 + 64, h // 2, 128 * lb:128 * lb + 128]

            def emit_A(hp):
                """rel scores for head pair (2hp, 2hp+1) -> DRAM bf16."""
                for h in (2 * hp, 2 * hp + 1):
                    for lb in range(7):   # zero-fill col-block 0 of rows 0..895
                        nc.sync.dma_start(
                            out=rbufs[h][128 * lb:128 * lb + 128, 0:128], in_=zero_bf)
                for lb in range(NB):
                    W = 128 * (lb + 1)
                    m0 = L - W
                    pss = []
                    for h in (2 * hp, 2 * hp + 1):   # paired emission: row groups 0-63 / 64-127
                        p0 = 64 * (h % 2)
                        ps = psA.tile([128, L], F32, tag="big")
                        pss.append(ps)
                        for (ns, wid) in _nsplit(W):
                            nc.tensor.matmul(
                                ps[:, ns:ns + wid],
                                qh(h, lb),
                                ft_sb[p0:p0 + 64, m0 + ns:m0 + ns + wid],
                                start=True, stop=True,
                            )
                    for i, h in enumerate((2 * hp, 2 * hp + 1)):
                        r_sb = rwork.tile([128, L], BF16, tag="r")
                        nc.vector.tensor_copy(r_sb[:, 0:W], pss[i][:, 0:W])
                        nc.sync.dma_start(
                            out=rbufs[h][128 * lb:128 * lb + 128, m0:L],
                            in_=r_sb[:, 0:W])

            def emit_B(hp):
                """logits, softmax, transpose, PV for head pair (2hp, 2hp+1)."""
                h0, h1 = 2 * hp, 2 * hp + 1
                # attnT ring: last 4 l-blocks' transposed columns per head
                atn = {h0: attnT_sb[:, :, 0:512], h1: attnT_sb[:, :, 512:1024]}
                for lb in range(NB):
                    W = 128 * (lb + 1)
                    chunks = _nsplit(W)
                    skews, pss = {}, {}
                    for h in (h0, h1):
                        skew = swork.tile([128, L], BF16, tag="skew")
                        nc.sync.dma_start(
                            out=skew[:, 0:W],
                            in_=bass.AP(tensor=rbufs[h],
                                        offset=128 * lb * 1023 + 1023,
                                        ap=[[1023, 128], [1, W]]),
                        )
                        skews[h] = skew
                    for h in (h0, h1):   # paired scores
                        p0 = 64 * (h % 2)
                        ps = psA.tile([128, L], F32, tag="big")
                        pss[h] = ps
                        for (ns, wid) in chunks:
                            nc.tensor.matmul(
                                ps[:, ns:ns + wid],
                                qh(h, lb),
                                kt_sb[p0:p0 + 64, h // 2, ns:ns + wid],
                                start=True, stop=False,
                            )
                    for h in (h0, h1):
                        ps = pss[h]
                        for (ns, wid) in chunks:
                            nc.tensor.matmul(
                                ps[:, ns:ns + wid], ident_bf, skews[h][:, ns:ns + wid],
                                start=False, stop=False, skip_group_check=True,
                            )
                        nc.tensor.matmul(
                            ps[:, W - 128:W], ident_bf, negtriu_bf,
                            start=False, stop=True, skip_group_check=True,
                        )
                    for h in (h0, h1):
                        ps = pss[h]
                        p_sb = pwork.tile([128, L], F32, tag="p")
                        sums = tiny.tile([128, 1], F32, tag="sums")
                        nc.scalar.activation(
                            p_sb[:, 0:W], ps[:, 0:W],
                            mybir.ActivationFunctionType.Exp,
                            scale=0.125, accum_out=sums,
                        )
                        rec = tiny.tile([128, 1], F32, tag="rec")
                        nc.vector.reciprocal(rec, sums)
                        attn_sb = awork.tile([128, L], F32, tag="attn")
                        nc.gpsimd.tensor_scalar_mul(attn_sb[:, 0:W], p_sb[:, 0:W], rec)
                        nc.sync.dma_start(
                            out=attn_d[h, 128 * lb:128 * lb + 128, 0:W],
                            in_=attn_sb[:, 0:W])
                        # transpose blocks jb=0..lb in groups of 4 into one PSUM bank
                        for g0 in range(0, lb + 1, 4):
                            gn = min(4, lb + 1 - g0)
                            pst = psT.tile([128, 512], F32, tag="t")
                            for i in range(gn):
                                jb = g0 + i
                                nc.tensor.transpose(
                                    pst[:, 128 * i:128 * i + 128],
                                    attn_sb[:, 128 * jb:128 * jb + 128],
                                    ident_f,
                                )
                            dst = atn[h][:, g0:g0 + gn, 128 * (lb % 4):128 * (lb % 4) + 128]
                            src = pst[:, 0:128 * gn].rearrange("p (g c) -> p g c", g=gn)
                            nc.scalar.activation(
                                dst, src, mybir.ActivationFunctionType.Copy)
                    if lb % 4 == 3:   # PV over the 4 buffered l-blocks
                        g = lb // 4          # 512-col group
                        base = 512 * g
                        jbs = [jb for jb in range(NB) if 128 * jb < base + 512]
                        for idx, h in enumerate((h0, h1)):
                            pso = psO.tile([64, 512], F32, tag="o")
                            for i, jb in enumerate(jbs):
                                lo = max(base, 128 * jb)
                                wid = base + 512 - lo
                                nc.tensor.matmul(
                                    pso[:, lo - base:lo - base + wid],
                                    v_sb[:, jb, HD * h:HD * h + HD],
                                    atn[h][:, jb, lo - base:lo - base + wid],
                                    start=(jb == 0), stop=(i == len(jbs) - 1),
                                    skip_group_check=True,
                                )
                            p0 = 64 * (h % 2)
                            nc.scalar.copy(
                                outT_sb[p0:p0 + 64, hp, 512 * g:512 * g + 512], pso)

            # interleave: A(0) A(1) B(0) A(2) B(1) A(3) B(2) B(3)
            emit_A(0)
            emit_A(1)
            emit_B(0)
            emit_A(2)
            emit_B(1)
            emit_A(3)
            emit_B(2)
            emit_B(3)

            # ---- O-projection ----
            for lb in range(NB):
                ps = psA.tile([128, D], F32, tag="big")
                for kc in range(KC):
                    nc.tensor.matmul(
                        ps,
                        outT_sb[:, kc, 128 * lb:128 * lb + 128],
                        wo_sb[:, kc, :],
                        start=(kc == 0), stop=(kc == KC - 1),
                    )
                o_sb = awork.tile([128, D], F32, tag="osb")
                nc.vector.scalar_tensor_tensor(
                    o_sb, ps, 1.0, bob,
                    op0=mybir.AluOpType.mult, op1=mybir.AluOpType.add,
                )
                nc.sync.dma_start(out=out_d[128 * lb:128 * lb + 128, :], in_=o_sb)

    nc.finalize()
    return nc


def _prep_core_inputs(inputs):
    """Host-side prep shared across cores (weights/constants)."""
    f32 = np.float32
    Wq = np.asarray(inputs["Wq"], f32)
    Wk = np.asarray(inputs["Wk"], f32)
    Wv = np.asarray(inputs["Wv"], f32)
    Wo = np.asarray(inputs["Wo"], f32)

    def wlay(Wm):
        return np.ascontiguousarray(Wm.reshape(KC, 128, D).transpose(1, 0, 2))

    rel = np.asarray(inputs["rel_emb"], f32)          # [1024, 64]
    ft = rel.T                                         # [64, 1024]
    ft2 = np.ascontiguousarray(np.concatenate([ft, ft], axis=0))  # [128, 1024]

    common = {
        "wq": wlay(Wq), "wk": wlay(Wk), "wv": wlay(Wv), "wo": wlay(Wo),
        "bq2": np.ascontiguousarray(np.asarray(inputs["bq"], f32).reshape(KC, 128).T),
        "bk2": np.ascontiguousarray(np.asarray(inputs["bk"], f32).reshape(KC, 128).T),
        "bv1": np.asarray(inputs["bv"], f32),
        "bo1": np.asarray(inputs["bo"], f32),
        "ft2": ft2,
        "identbf": np.eye(128, dtype=ml_dtypes.bfloat16),
        "negtriubf": (np.triu(np.ones((128, 128), f32), k=1) * -24000.0).astype(ml_dtypes.bfloat16),
        "identf": np.eye(128, dtype=f32),
        "zerobf": np.zeros((128, 128), ml_dtypes.bfloat16),
    }
    return common


def xlay_pub(xb):   # [L, D] -> xT [512, 1024] -> [128, KC, L]
    xt = xb.T   # [D, L]
    return np.ascontiguousarray(xt.reshape(KC, 128, L).transpose(1, 0, 2))


def kernel(**inputs):
    if "nc" not in _cache:
        _cache["nc"] = build_program()
    nc = _cache["nc"]

    common = _prep_core_inputs(inputs)
    x_q = np.asarray(inputs["x_q"], np.float32)
    x_k = np.asarray(inputs["x_k"], np.float32)
    x_v = np.asarray(inputs["x_v"], np.float32)
    xlay = xlay_pub

    in_maps = []
    for b in range(B):
        m = dict(common)
        m["xqt"] = xlay(x_q[b])
        m["xkt"] = xlay(x_k[b])
        m["xvt"] = xlay(x_v[b])
        in_maps.append(m)

    res = run_bass_kernel_spmd(nc, in_maps, list(range(B)))
    out = np.stack([res.results[b]["out"] for b in range(B)])      # [8, 1024, 512]
    attn = np.stack([res.results[b]["attn"] for b in range(B)])    # [8, 8, 1024, 1024]
    return out, attn


# revision 11
# speedup vs baseline: 2.2222x; 2.2222x over previous
"""Trainium2 Bass kernel for EnhancedRelativeGlobalAttention (B=8, L=1024, D=512, H=8).

Strategy: data-parallel over batch (1 batch element per NeuronCore, 8 cores).
Per core:
  - QKV projections with PE (f32r matmuls), Q/K in transposed layout [D, L],
    V in natural layout [L, D].
  - Per head: rel scores R = q_h @ rel_emb^T computed triangularly, written to
    a flat DRAM scratch (bf16); the Music-Transformer "skew" becomes a strided
    DMA read of that flat buffer (offset 1023, row-step 1023).
  - Logits accumulated in PSUM: scores matmul + skew added via identity-matmul
    + causal mask added via identity-matmul of a -24000 triangular block.
    exp on ScalarE with fused per-row accumulation (softmax denominators).
  - Normalize on VectorE (per-partition reciprocal scalar) -> attn output,
    PE-transpose of attn blocks feeds the PV matmul; O-projection at the end.
  - Causal structure: strictly-upper 128x128 blocks are never computed; the
    output buffer is pre-zeroed by the runtime.
"""
import sys
import os

sys.path.insert(0, '/opt/trn_rl_repo')

import numpy as np
import ml_dtypes

import concourse.bass as bass
import concourse.mybir as mybir
import concourse.tile as tile
from concourse import bacc
from concourse.bass_utils import run_bass_kernel_spmd

F32 = mybir.dt.float32
BF16 = mybir.dt.bfloat16
F32R = mybir.dt.float32r

B, L, D, H = 8, 1024, 512, 8
HD = D // H          # 64
NB = L // 128        # 8 l-blocks
KC = D // 128        # 4 contraction chunks

_cache = {}


def _nsplit(W):
    """Split [0, W) into PSUM-bank-aligned matmul chunks (<=512 each)."""
    if W <= 512:
        return [(0, W)]
    return [(0, 512), (512, W - 512)]


def build_program():
    nc = bacc.Bacc("TRN2", target_bir_lowering=False, debug=False, num_devices=8)

    # ---- DRAM I/O ----
    xqt = nc.dram_tensor("xqt", [128, KC, L], F32R, kind="ExternalInput")
    xkt = nc.dram_tensor("xkt", [128, KC, L], F32R, kind="ExternalInput")
    xvt = nc.dram_tensor("xvt", [128, KC, L], F32R, kind="ExternalInput")
    wq = nc.dram_tensor("wq", [128, KC, D], F32R, kind="ExternalInput")
    wk = nc.dram_tensor("wk", [128, KC, D], F32R, kind="ExternalInput")
    wv = nc.dram_tensor("wv", [128, KC, D], F32R, kind="ExternalInput")
    wo = nc.dram_tensor("wo", [128, KC, D], F32R, kind="ExternalInput")
    bq2 = nc.dram_tensor("bq2", [128, KC], F32, kind="ExternalInput")
    bk2 = nc.dram_tensor("bk2", [128, KC], F32, kind="ExternalInput")
    bv1 = nc.dram_tensor("bv1", [D], F32, kind="ExternalInput")
    bo1 = nc.dram_tensor("bo1", [D], F32, kind="ExternalInput")
    ft2 = nc.dram_tensor("ft2", [128, L], F32R, kind="ExternalInput")   # rel_emb^T stacked twice
    identbf = nc.dram_tensor("identbf", [128, 128], BF16, kind="ExternalInput")
    negtriubf = nc.dram_tensor("negtriubf", [128, 128], BF16, kind="ExternalInput")
    identf = nc.dram_tensor("identf", [128, 128], F32, kind="ExternalInput")
    zerobf = nc.dram_tensor("zerobf", [128, 128], BF16, kind="ExternalInput")

    out_d = nc.dram_tensor("out", [L, D], F32, kind="ExternalOutput")
    attn_d = nc.dram_tensor("attn", [H, L, L], F32, kind="ExternalOutput")
    rbufs = [nc.dram_tensor(f"r{h}", [L, L], BF16) for h in range(H)]

    with tile.TileContext(nc) as tc:
        with (
            tc.tile_pool(name="consts", bufs=1) as consts,
            tc.tile_pool(name="persist", bufs=1) as persist,
            tc.tile_pool(name="psA", bufs=2, space="PSUM") as psA,
            tc.tile_pool(name="psT", bufs=2, space="PSUM") as psT,
            tc.tile_pool(name="psO", bufs=2, space="PSUM") as psO,
            tc.tile_pool(name="rwork", bufs=2) as rwork,
            tc.tile_pool(name="swork", bufs=2) as swork,
            tc.tile_pool(name="pwork", bufs=2) as pwork,
            tc.tile_pool(name="awork", bufs=2) as awork,
            tc.tile_pool(name="tiny", bufs=4) as tiny,
        ):
            # ---- constants ----
            ft_sb = consts.tile([128, L], F32R, tag="ft")
            nc.sync.dma_start(out=ft_sb, in_=ft2[:])
            ident_bf = consts.tile([128, 128], BF16, tag="identbf")
            nc.sync.dma_start(out=ident_bf, in_=identbf[:])
            negtriu_bf = consts.tile([128, 128], BF16, tag="negtriu")
            nc.sync.dma_start(out=negtriu_bf, in_=negtriubf[:])
            ident_f = consts.tile([128, 128], F32, tag="identf")
            nc.sync.dma_start(out=ident_f, in_=identf[:])
            zero_bf = consts.tile([128, 128], BF16, tag="zerobf")
            nc.sync.dma_start(out=zero_bf, in_=zerobf[:])
            bq_sb = consts.tile([128, KC], F32, tag="bq")
            nc.sync.dma_start(out=bq_sb, in_=bq2[:])
            bk_sb = consts.tile([128, KC], F32, tag="bk")
            nc.sync.dma_start(out=bk_sb, in_=bk2[:])
            bvb = consts.tile([128, D], F32, tag="bvb")
            nc.sync.dma_start(out=bvb, in_=bass.AP(tensor=bv1, offset=0, ap=[[0, 128], [1, D]]))
            bob = consts.tile([128, D], F32, tag="bob")
            nc.sync.dma_start(out=bob, in_=bass.AP(tensor=bo1, offset=0, ap=[[0, 128], [1, D]]))

            # ---- persistent activations ----
            qt_sb = persist.tile([128, KC, L], F32R, tag="qt")      # q^T  [D, L]
            kt_sb = persist.tile([128, KC, L], F32R, tag="kt")      # k^T  [D, L]
            v_sb = persist.tile([128, NB, D], F32R, tag="v")        # v natural [L, D]
            wo_sb = persist.tile([128, KC, D], F32R, tag="wo")
            outT_sb = persist.tile([128, KC, L], F32R, tag="outT")  # attn_out^T [D, L]
            attnT_sb = persist.tile([128, NB, L], F32R, tag="attnT")  # ring: [j, jb, 2x512 l-cols]
            nc.sync.dma_start(out=wo_sb, in_=wo[:])

            # ---- projections ----
            with (
                tc.tile_pool(name="xbuf", bufs=2) as xbuf,
                tc.tile_pool(name="wstream", bufs=8) as wstream,
                tc.tile_pool(name="wvres", bufs=1) as wvres,
            ):
                # qT / kT: out[dchunk][128, L] = sum_kc W[kc][:, dcols].T @ xT[kc]
                for (xd, wd, dst, b_sb) in (
                    (xqt, wq, qt_sb, bq_sb),
                    (xkt, wk, kt_sb, bk_sb),
                ):
                    x_sb = xbuf.tile([128, KC, L], F32R, tag="x")
                    nc.sync.dma_start(out=x_sb, in_=xd[:])
                    for c in range(KC):
                        ps = psA.tile([128, L], F32, tag="big")
                        for kc in range(KC):
                            wch = wstream.tile([128, 128], F32R, tag="wch")
                            nc.sync.dma_start(
                                out=wch, in_=wd[:, kc, 128 * c:128 * c + 128])
                            for (ns, wid) in _nsplit(L):
                                nc.tensor.matmul(
                                    ps[:, ns:ns + wid],
                                    wch,
                                    x_sb[:, kc, ns:ns + wid],
                                    start=(kc == 0),
                                    stop=(kc == KC - 1),
                                )
                        nc.scalar.activation(
                            dst[:, c, :], ps,
                            mybir.ActivationFunctionType.Identity,
                            bias=b_sb[:, c:c + 1], scale=1.0,
                        )
                # v natural: v[jb][128, D] = sum_kc xvT[kc][:, jcols].T @ Wv[kc]
                xv_sb = xbuf.tile([128, KC, L], F32R, tag="x")
                nc.sync.dma_start(out=xv_sb, in_=xvt[:])
                wv_sb = wvres.tile([128, KC, D], F32R, tag="wv")
                nc.sync.dma_start(out=wv_sb, in_=wv[:])
                for jb in range(NB):
                    ps = psA.tile([128, D], F32, tag="big")
                    for kc in range(KC):
                        nc.tensor.matmul(
                            ps,
                            xv_sb[:, kc, 128 * jb:128 * jb + 128],
                            wv_sb[:, kc, :],
                            start=(kc == 0),
                            stop=(kc == KC - 1),
                        )
                    nc.vector.scalar_tensor_tensor(
                        v_sb[:, jb, :], ps, 1.0, bvb,
                        op0=mybir.AluOpType.mult, op1=mybir.AluOpType.add,
                    )

            def qh(h, lb):
                """lhsT slice of q^T for head h, l-block lb: [64, 128]."""
                p0 = 64 * (h % 2)
                return qt_sb[p0:p0 + 64, h // 2, 128 * lb:128 * lb + 128]

            def emit_A(hp):
                """rel scores for head pair (2hp, 2hp+1) -> DRAM bf16."""
                for h in (2 * hp, 2 * hp + 1):
                    for lb in range(7):   # zero-fill col-block 0 of rows 0..895
                        nc.sync.dma_start(
                            out=rbufs[h][128 * lb:128 * lb + 128, 0:128], in_=zero_bf)
                for lb in range(NB):
                    W = 128 * (lb + 1)
                    m0 = L - W
                    pss = []
                    for h in (2 * hp, 2 * hp + 1):   # paired emission: row groups 0-63 / 64-127
                        p0 = 64 * (h % 2)
                        ps = psA.tile([128, L], F32, tag="big")
                        pss.append(ps)
                        for (ns, wid) in _nsplit(W):
                            nc.tensor.matmul(
                                ps[:, ns:ns + wid],
                                qh(h, lb),
                                ft_sb[p0:p0 + 64, m0 + ns:m0 + ns + wid],
                                start=True, stop=True,
                                tile_position=(p0, 0),
                            )
                    for i, h in enumerate((2 * hp, 2 * hp + 1)):
                        r_sb = rwork.tile([128, L], BF16, tag="r")
                        nc.vector.tensor_copy(r_sb[:, 0:W], pss[i][:, 0:W])
                        nc.sync.dma_start(
                            out=rbufs[h][128 * lb:128 * lb + 128, m0:L],
                            in_=r_sb[:, 0:W])

            def emit_B(hp):
                """logits, softmax, transpose, PV for head pair (2hp, 2hp+1)."""
                h0, h1 = 2 * hp, 2 * hp + 1
                # attnT ring: last 4 l-blocks' transposed columns per head
                atn = {h0: attnT_sb[:, :, 0:512], h1: attnT_sb[:, :, 512:1024]}
                for lb in range(NB):
                    W = 128 * (lb + 1)
                    chunks = _nsplit(W)
                    skews, pss = {}, {}
                    for h in (h0, h1):
                        skew = swork.tile([128, L], BF16, tag="skew")
                        nc.sync.dma_start(
                            out=skew[:, 0:W],
                            in_=bass.AP(tensor=rbufs[h],
                                        offset=128 * lb * 1023 + 1023,
                                        ap=[[1023, 128], [1, W]]),
                        )
                        skews[h] = skew
                    for h in (h0, h1):   # paired scores
                        p0 = 64 * (h % 2)
                        ps = psA.tile([128, L], F32, tag="big")
                        pss[h] = ps
                        for (ns, wid) in chunks:
                            nc.tensor.matmul(
                                ps[:, ns:ns + wid],
                                qh(h, lb),
                                kt_sb[p0:p0 + 64, h // 2, ns:ns + wid],
                                start=True, stop=False,
                                tile_position=(p0, 0),
                            )
                    for h in (h0, h1):
                        ps = pss[h]
                        for (ns, wid) in chunks:
                            nc.tensor.matmul(
                                ps[:, ns:ns + wid], ident_bf, skews[h][:, ns:ns + wid],
                                start=False, stop=False, skip_group_check=True,
                            )
                        nc.tensor.matmul(
                            ps[:, W - 128:W], ident_bf, negtriu_bf,
                            start=False, stop=True, skip_group_check=True,
                        )
                    for h in (h0, h1):
                        ps = pss[h]
                        p_sb = pwork.tile([128, L], F32, tag="p")
                        sums = tiny.tile([128, 1], F32, tag="sums")
                        nc.scalar.activation(
                            p_sb[:, 0:W], ps[:, 0:W],
                            mybir.ActivationFunctionType.Exp,
                            scale=0.125, accum_out=sums,
                        )
                        rec = tiny.tile([128, 1], F32, tag="rec")
                        nc.vector.reciprocal(rec, sums)
                        attn_sb = awork.tile([128, L], F32, tag="attn")
                        nc.vector.tensor_scalar_mul(attn_sb[:, 0:W], p_sb[:, 0:W], rec)
                        nc.sync.dma_start(
                            out=attn_d[h, 128 * lb:128 * lb + 128, 0:W],
                            in_=attn_sb[:, 0:W])
                        # transpose blocks jb=0..lb in groups of 4 into one PSUM bank
                        for g0 in range(0, lb + 1, 4):
                            gn = min(4, lb + 1 - g0)
                            pst = psT.tile([128, 512], F32, tag="t")
                            for i in range(gn):
                                jb = g0 + i
                                nc.tensor.transpose(
                                    pst[:, 128 * i:128 * i + 128],
                                    attn_sb[:, 128 * jb:128 * jb + 128],
                                    ident_f,
                                )
                            dst = atn[h][:, g0:g0 + gn, 128 * (lb % 4):128 * (lb % 4) + 128]
                            src = pst[:, 0:128 * gn].rearrange("p (g c) -> p g c", g=gn)
                            nc.scalar.activation(
                                dst, src, mybir.ActivationFunctionType.Copy)
                    if lb % 4 == 3:   # PV over the 4 buffered l-blocks
                        g = lb // 4          # 512-col group
                        base = 512 * g
                        jbs = [jb for jb in range(NB) if 128 * jb < base + 512]
                        for idx, h in enumerate((h0, h1)):
                            pso = psO.tile([64, 512], F32, tag="o")
                            for i, jb in enumerate(jbs):
                                lo = max(base, 128 * jb)
                                wid = base + 512 - lo
                                nc.tensor.matmul(
                                    pso[:, lo - base:lo - base + wid],
                                    v_sb[:, jb, HD * h:HD * h + HD],
                                    atn[h][:, jb, lo - base:lo - base + wid],
                                    start=(jb == 0), stop=(i == len(jbs) - 1),
                                    skip_group_check=True,
                                )
                            p0 = 64 * (h % 2)
                            nc.scalar.copy(
                                outT_sb[p0:p0 + 64, hp, 512 * g:512 * g + 512], pso)

            # interleave: A(0) A(1) B(0) A(2) B(1) A(3) B(2) B(3)
            emit_A(0)
            emit_A(1)
            emit_B(0)
            emit_A(2)
            emit_B(1)
            emit_A(3)
            emit_B(2)
            emit_B(3)

            # ---- O-projection ----
            for lb in range(NB):
                ps = psA.tile([128, D], F32, tag="big")
                for kc in range(KC):
                    nc.tensor.matmul(
                        ps,
                        outT_sb[:, kc, 128 * lb:128 * lb + 128],
                        wo_sb[:, kc, :],
                        start=(kc == 0), stop=(kc == KC - 1),
                    )
                o_sb = awork.tile([128, D], F32, tag="osb")
                nc.vector.scalar_tensor_tensor(
                    o_sb, ps, 1.0, bob,
                    op0=mybir.AluOpType.mult, op1=mybir.AluOpType.add,
                )
                nc.sync.dma_start(out=out_d[128 * lb:128 * lb + 128, :], in_=o_sb)

    nc.finalize()
    return nc


def _prep_core_inputs(inputs):
    """Host-side prep shared across cores (weights/constants)."""
    f32 = np.float32
    Wq = np.asarray(inputs["Wq"], f32)
    Wk = np.asarray(inputs["Wk"], f32)
    Wv = np.asarray(inputs["Wv"], f32)
    Wo = np.asarray(inputs["Wo"], f32)

    def wlay(Wm):
        return np.ascontiguousarray(Wm.reshape(KC, 128, D).transpose(1, 0, 2))

    rel = np.asarray(inputs["rel_emb"], f32)          # [1024, 64]
    ft = rel.T                                         # [64, 1024]
    ft2 = np.ascontiguousarray(np.concatenate([ft, ft], axis=0))  # [128, 1024]

    common = {
        "wq": wlay(Wq), "wk": wlay(Wk), "wv": wlay(Wv), "wo": wlay(Wo),
        "bq2": np.ascontiguousarray(np.asarray(inputs["bq"], f32).reshape(KC, 128).T),
        "bk2": np.ascontiguousarray(np.asarray(inputs["bk"], f32).reshape(KC, 128).T),
        "bv1": np.asarray(inputs["bv"], f32),
        "bo1": np.asarray(inputs["bo"], f32),
        "ft2": ft2,
        "identbf": np.eye(128, dtype=ml_dtypes.bfloat16),
        "negtriubf": (np.triu(np.ones((128, 128), f32), k=1) * -24000.0).astype(ml_dtypes.bfloat16),
        "identf": np.eye(128, dtype=f32),
        "zerobf": np.zeros((128, 128), ml_dtypes.bfloat16),
    }
    return common


def xlay_pub(xb):   # [L, D] -> xT [512, 1024] -> [128, KC, L]
    xt = xb.T   # [D, L]
    return np.ascontiguousarray(xt.reshape(KC, 128, L).transpose(1, 0, 2))


def kernel(**inputs):
    if "nc" not in _cache:
        _cache["nc"] = build_program()
    nc = _cache["nc"]

    common = _prep_core_inputs(inputs)
    x_q = np.asarray(inputs["x_q"], np.float32)
    x_k = np.asarray(inputs["x_k"], np.float32)
    x_v = np.asarray(inputs["x_v"], np.float32)
    xlay = xlay_pub

    in_maps = []
    for b in range(B):
        m = dict(common)
        m["xqt"] = xlay(x_q[b])
        m["xkt"] = xlay(x_k[b])
        m["xvt"] = xlay(x_v[b])
        in_maps.append(m)

    res = run_bass_kernel_spmd(nc, in_maps, list(range(B)))
    out = np.stack([res.results[b]["out"] for b in range(B)])      # [8, 1024, 512]
    attn = np.stack([res.results[b]["attn"] for b in range(B)])    # [8, 8, 1024, 1024]
    return out, attn


# revision 15
# speedup vs baseline: 2.4478x; 1.1015x over previous
"""Trainium2 Bass kernel for EnhancedRelativeGlobalAttention (B=8, L=1024, D=512, H=8).

Strategy: data-parallel over batch (1 batch element per NeuronCore, 8 cores).
Per core:
  - QKV projections with PE (f32r matmuls), Q/K in transposed layout [D, L],
    V in natural layout [L, D].
  - Per head: rel scores R = q_h @ rel_emb^T computed triangularly, written to
    a flat DRAM scratch (bf16); the Music-Transformer "skew" becomes a strided
    DMA read of that flat buffer (offset 1023, row-step 1023).
  - Logits accumulated in PSUM: scores matmul + skew added via identity-matmul
    + causal mask added via identity-matmul of a -24000 triangular block.
    exp on ScalarE with fused per-row accumulation (softmax denominators).
  - Normalize on VectorE (per-partition reciprocal scalar) -> attn output,
    PE-transpose of attn blocks feeds the PV matmul; O-projection at the end.
  - Causal structure: strictly-upper 128x128 blocks are never computed; the
    output buffer is pre-zeroed by the runtime.
"""
import sys
import os

sys.path.insert(0, '/opt/trn_rl_repo')

import numpy as np
import ml_dtypes

import concourse.bass as bass
import concourse.mybir as mybir
import concourse.tile as tile
from concourse import bacc
from concourse.bass_utils import run_bass_kernel_spmd

F32 = mybir.dt.float32
BF16 = mybir.dt.bfloat16
F32R = mybir.dt.float32r

B, L, D, H = 8, 1024, 512, 8
HD = D // H          # 64
NB = L // 128        # 8 l-blocks
KC = D // 128        # 4 contraction chunks

_cache = {}


def _nsplit(W):
    """Split [0, W) into PSUM-bank-aligned matmul chunks (<=512 each)."""
    if W <= 512:
        return [(0, W)]
    return [(0, 512), (512, W - 512)]


def build_program():
    nc = bacc.Bacc("TRN2", target_bir_lowering=False, debug=False, num_devices=8)

    # ---- DRAM I/O ----
    xqt = nc.dram_tensor("xqt", [128, KC, L], F32R, kind="ExternalInput")
    xkt = nc.dram_tensor("xkt", [128, KC, L], F32R, kind="ExternalInput")
    xvt = nc.dram_tensor("xvt", [128, KC, L], F32R, kind="ExternalInput")
    wq = nc.dram_tensor("wq", [128, KC, D], F32R, kind="ExternalInput")
    wk = nc.dram_tensor("wk", [128, KC, D], F32R, kind="ExternalInput")
    wv = nc.dram_tensor("wv", [128, KC, D], F32R, kind="ExternalInput")
    wo = nc.dram_tensor("wo", [128, KC, D], F32R, kind="ExternalInput")
    bq2 = nc.dram_tensor("bq2", [128, KC], F32, kind="ExternalInput")
    bk2 = nc.dram_tensor("bk2", [128, KC], F32, kind="ExternalInput")
    bv1 = nc.dram_tensor("bv1", [D], F32, kind="ExternalInput")
    bo1 = nc.dram_tensor("bo1", [D], F32, kind="ExternalInput")
    ft2 = nc.dram_tensor("ft2", [128, L], F32R, kind="ExternalInput")   # rel_emb^T stacked twice
    identbf = nc.dram_tensor("identbf", [128, 128], BF16, kind="ExternalInput")
    negtriubf = nc.dram_tensor("negtriubf", [128, 128], BF16, kind="ExternalInput")
    identf = nc.dram_tensor("identf", [128, 128], F32, kind="ExternalInput")
    zerobf = nc.dram_tensor("zerobf", [128, 128], BF16, kind="ExternalInput")

    out_d = nc.dram_tensor("out", [L, D], F32, kind="ExternalOutput")
    attn_d = nc.dram_tensor("attn", [H, L, L], F32, kind="ExternalOutput")
    rbufs = [nc.dram_tensor(f"r{h}", [L, L], BF16) for h in range(H)]

    with tile.TileContext(nc) as tc:
        with (
            tc.tile_pool(name="consts", bufs=1) as consts,
            tc.tile_pool(name="persist", bufs=1) as persist,
            tc.tile_pool(name="psA", bufs=2, space="PSUM") as psA,
            tc.tile_pool(name="psT", bufs=2, space="PSUM") as psT,
            tc.tile_pool(name="psO", bufs=2, space="PSUM") as psO,
            tc.tile_pool(name="rwork", bufs=2) as rwork,
            tc.tile_pool(name="swork", bufs=2) as swork,
            tc.tile_pool(name="pwork", bufs=2) as pwork,
            tc.tile_pool(name="awork", bufs=2) as awork,
            tc.tile_pool(name="tiny", bufs=4) as tiny,
        ):
            # ---- constants ----
            ft_sb = consts.tile([128, L], F32R, tag="ft")
            nc.sync.dma_start(out=ft_sb, in_=ft2[:])
            ident_bf = consts.tile([128, 128], BF16, tag="identbf")
            nc.sync.dma_start(out=ident_bf, in_=identbf[:])
            negtriu_bf = consts.tile([128, 128], BF16, tag="negtriu")
            nc.sync.dma_start(out=negtriu_bf, in_=negtriubf[:])
            ident_f = consts.tile([128, 128], F32, tag="identf")
            nc.sync.dma_start(out=ident_f, in_=identf[:])
            negfill_bf = consts.tile([128, 128], BF16, tag="negfill")
            nc.sync.dma_start(out=negfill_bf, in_=zerobf[:])
            bq_sb = consts.tile([128, KC], F32, tag="bq")
            nc.sync.dma_start(out=bq_sb, in_=bq2[:])
            bk_sb = consts.tile([128, KC], F32, tag="bk")
            nc.sync.dma_start(out=bk_sb, in_=bk2[:])
            bvb = consts.tile([128, D], F32, tag="bvb")
            nc.sync.dma_start(out=bvb, in_=bass.AP(tensor=bv1, offset=0, ap=[[0, 128], [1, D]]))
            bob = consts.tile([128, D], F32, tag="bob")
            nc.sync.dma_start(out=bob, in_=bass.AP(tensor=bo1, offset=0, ap=[[0, 128], [1, D]]))

            # ---- persistent activations ----
            qt_sb = persist.tile([128, KC, L], F32R, tag="qt")      # q^T  [D, L]
            kt_sb = persist.tile([128, KC, L], F32R, tag="kt")      # k^T  [D, L]
            v_sb = persist.tile([128, NB, D], F32R, tag="v")        # v natural [L, D]
            wo_sb = persist.tile([128, KC, D], F32R, tag="wo")
            outT_sb = persist.tile([128, KC, L], F32R, tag="outT")  # attn_out^T [D, L]
            attnT_sb = persist.tile([128, NB, L], F32R, tag="attnT")  # ring: [j, jb, 2x512 l-cols]
            nc.sync.dma_start(out=wo_sb, in_=wo[:])

            # ---- projections ----
            with (
                tc.tile_pool(name="xbuf", bufs=2) as xbuf,
                tc.tile_pool(name="wstream", bufs=8) as wstream,
                tc.tile_pool(name="wvres", bufs=1) as wvres,
            ):
                # qT / kT: out[dchunk][128, L] = sum_kc W[kc][:, dcols].T @ xT[kc]
                for (xd, wd, dst, b_sb) in (
                    (xqt, wq, qt_sb, bq_sb),
                    (xkt, wk, kt_sb, bk_sb),
                ):
                    x_sb = xbuf.tile([128, KC, L], F32R, tag="x")
                    for kc in range(KC):
                        nc.sync.dma_start(out=x_sb[:, kc, :], in_=xd[:, kc, :])
                    for c in range(KC):
                        ps = psA.tile([128, L], F32, tag="big")
                        for kc in range(KC):
                            wch = wstream.tile([128, 128], F32R, tag="wch")
                            nc.sync.dma_start(
                                out=wch, in_=wd[:, kc, 128 * c:128 * c + 128])
                            for (ns, wid) in _nsplit(L):
                                nc.tensor.matmul(
                                    ps[:, ns:ns + wid],
                                    wch,
                                    x_sb[:, kc, ns:ns + wid],
                                    start=(kc == 0),
                                    stop=(kc == KC - 1),
                                )
                        nc.scalar.activation(
                            dst[:, c, :], ps,
                            mybir.ActivationFunctionType.Identity,
                            bias=b_sb[:, c:c + 1], scale=1.0,
                        )
                # v natural: v[jb][128, D] = sum_kc xvT[kc][:, jcols].T @ Wv[kc]
                xv_sb = xbuf.tile([128, KC, L], F32R, tag="x")
                for kc in range(KC):
                    nc.sync.dma_start(out=xv_sb[:, kc, :], in_=xvt[:, kc, :])
                wv_sb = wvres.tile([128, KC, D], F32R, tag="wv")
                nc.sync.dma_start(out=wv_sb, in_=wv[:])
                for jb in range(NB):
                    ps = psA.tile([128, D], F32, tag="big")
                    for kc in range(KC):
                        nc.tensor.matmul(
                            ps,
                            xv_sb[:, kc, 128 * jb:128 * jb + 128],
                            wv_sb[:, kc, :],
                            start=(kc == 0),
                            stop=(kc == KC - 1),
                        )
                    nc.vector.scalar_tensor_tensor(
                        v_sb[:, jb, :], ps, 1.0, bvb,
                        op0=mybir.AluOpType.mult, op1=mybir.AluOpType.add,
                    )

            def qh(h, lb):
                """lhsT slice of q^T for head h, l-block lb: [64, 128]."""
                p0 = 64 * (h % 2)
                return qt_sb[p0:p0 + 64, h // 2, 128 * lb:128 * lb + 128]

            def emit_A_fill(hp):
                for h in (2 * hp, 2 * hp + 1):
                    for lb in range(7):   # mask-fill col-block 0 of rows 0..895
                        nc.sync.dma_start(
                            out=rbufs[h][128 * lb:128 * lb + 128, 0:128], in_=negfill_bf)

            def emit_A_lb(hp, lb):
                """rel scores for head pair (2hp, 2hp+1), one l-block -> DRAM bf16."""
                W = 128 * (lb + 1)
                m0 = L - W
                pss = []
                for h in (2 * hp, 2 * hp + 1):   # paired: row groups 0-63 / 64-127
                    p0 = 64 * (h % 2)
                    ps = psA.tile([128, L], F32, tag="big")
                    pss.append(ps)
                    for (ns, wid) in _nsplit(W):
                        nc.tensor.matmul(
                            ps[:, ns:ns + wid],
                            qh(h, lb),
                            ft_sb[p0:p0 + 64, m0 + ns:m0 + ns + wid],
                            start=True, stop=True,
                            tile_position=(p0, 0),
                        )
                for i, h in enumerate((2 * hp, 2 * hp + 1)):
                    r_sb = rwork.tile([128, L], BF16, tag="r")
                    nc.vector.tensor_copy(r_sb[:, 0:W], pss[i][:, 0:W])
                    nc.sync.dma_start(
                        out=rbufs[h][128 * lb:128 * lb + 128, m0:L],
                        in_=r_sb[:, 0:W])

            def emit_B_lb(hp, lb, atn):
                """logits, softmax, transpose (+PV at lb%4==3) for one l-block."""
                h0, h1 = 2 * hp, 2 * hp + 1
                W = 128 * (lb + 1)
                chunks = _nsplit(W)
                skews, pss = {}, {}
                for h in (h0, h1):
                    skew = swork.tile([128, L], BF16, tag="skew")
                    nc.sync.dma_start(
                        out=skew[:, 0:W],
                        in_=bass.AP(tensor=rbufs[h],
                                    offset=128 * lb * 1023 + 1023,
                                    ap=[[1023, 128], [1, W]]),
                    )
                    skews[h] = skew
                for h in (h0, h1):   # skew-add first: psum claimed when skew arrives
                    ps = psA.tile([128, L], F32, tag="big")
                    pss[h] = ps
                    for (ns, wid) in chunks:
                        nc.tensor.matmul(
                            ps[:, ns:ns + wid], ident_bf, skews[h][:, ns:ns + wid],
                            start=True, stop=False, skip_group_check=True,
                        )
                for h in (h0, h1):   # paired scores
                    p0 = 64 * (h % 2)
                    ps = pss[h]
                    for ci, (ns, wid) in enumerate(chunks):
                        nc.tensor.matmul(
                            ps[:, ns:ns + wid],
                            qh(h, lb),
                            kt_sb[p0:p0 + 64, h // 2, ns:ns + wid],
                            start=False, stop=((lb != NB - 1) or ns == 0),
                            skip_group_check=True,
                            tile_position=(p0, 0),
                        )
                    if lb == NB - 1:   # only the last l-block needs the mask matmul
                        nc.tensor.matmul(
                            ps[:, W - 128:W], ident_bf, negtriu_bf,
                            start=False, stop=True, skip_group_check=True,
                        )
                for h in (h0, h1):
                    ps = pss[h]
                    p_sb = pwork.tile([128, L], F32, tag="p")
                    sums = tiny.tile([128, 1], F32, tag="sums")
                    nc.scalar.activation(
                        p_sb[:, 0:W], ps[:, 0:W],
                        mybir.ActivationFunctionType.Exp,
                        scale=0.125, accum_out=sums,
                    )
                    rec = tiny.tile([128, 1], F32, tag="rec")
                    nc.vector.reciprocal(rec, sums)
                    attn_sb = awork.tile([128, L], F32, tag="attn")
                    nc.vector.tensor_scalar_mul(attn_sb[:, 0:W], p_sb[:, 0:W], rec)
                    nc.sync.dma_start(
                        out=attn_d[h, 128 * lb:128 * lb + 128, 0:W],
                        in_=attn_sb[:, 0:W])
                    # transpose blocks jb=0..lb in groups of 4 into one PSUM bank
                    for g0 in range(0, lb + 1, 4):
                        gn = min(4, lb + 1 - g0)
                        pst = psT.tile([128, 512], F32, tag="t")
                        for i in range(gn):
                            jb = g0 + i
                            nc.tensor.transpose(
                                pst[:, 128 * i:128 * i + 128],
                                attn_sb[:, 128 * jb:128 * jb + 128],
                                ident_f,
                            )
                        dst = atn[h][:, g0:g0 + gn, 128 * (lb % 4):128 * (lb % 4) + 128]
                        src = pst[:, 0:128 * gn].rearrange("p (g c) -> p g c", g=gn)
                        if (lb + g0 // 4) % 2 == 0:
                            nc.scalar.activation(
                                dst, src, mybir.ActivationFunctionType.Copy)
                        else:
                            nc.vector.tensor_copy(dst, src)
                if lb % 4 == 3:   # PV over the 4 buffered l-blocks
                    g = lb // 4          # 512-col group
                    base = 512 * g
                    jbs = [jb for jb in range(NB) if 128 * jb < base + 512]
                    for idx, h in enumerate((h0, h1)):
                        pso = psO.tile([64, 512], F32, tag="o")
                        for i, jb in enumerate(jbs):
                            lo = max(base, 128 * jb)
                            wid = base + 512 - lo
                            nc.tensor.matmul(
                                pso[:, lo - base:lo - base + wid],
                                v_sb[:, jb, HD * h:HD * h + HD],
                                atn[h][:, jb, lo - base:lo - base + wid],
                                start=(jb == 0), stop=(i == len(jbs) - 1),
                                skip_group_check=True,
                            )
                        p0 = 64 * (h % 2)
                        nc.scalar.copy(
                            outT_sb[p0:p0 + 64, hp, 512 * g:512 * g + 512], pso)

            def make_atn(hp):
                return {2 * hp: attnT_sb[:, :, 0:512],
                        2 * hp + 1: attnT_sb[:, :, 512:1024]}

            def emit_A(hp):
                emit_A_fill(hp)
                for lb in range(NB):
                    emit_A_lb(hp, lb)

            def emit_B(hp):
                atn = make_atn(hp)
                for lb in range(NB):
                    emit_B_lb(hp, lb, atn)

            def emit_AB(ahp, bhp):
                emit_A_fill(ahp)
                atn = make_atn(bhp)
                for lb in range(NB):
                    emit_A_lb(ahp, lb)
                    emit_B_lb(bhp, lb, atn)

            # software pipeline: A(0); then A(k+1) interleaved with B(k) per lb
            emit_A(0)
            for k in range(4):
                if k + 1 < 4:
                    emit_AB(k + 1, k)
                else:
                    emit_B(k)

            # ---- O-projection ----
            for lb in range(NB):
                ps = psA.tile([128, D], F32, tag="big")
                for kc in range(KC):
                    nc.tensor.matmul(
                        ps,
                        outT_sb[:, kc, 128 * lb:128 * lb + 128],
                        wo_sb[:, kc, :],
                        start=(kc == 0), stop=(kc == KC - 1),
                    )
                o_sb = awork.tile([128, D], F32, tag="osb")
                nc.vector.scalar_tensor_tensor(
                    o_sb, ps, 1.0, bob,
                    op0=mybir.AluOpType.mult, op1=mybir.AluOpType.add,
                )
                nc.sync.dma_start(out=out_d[128 * lb:128 * lb + 128, :], in_=o_sb)

    nc.finalize()
    return nc


def _prep_core_inputs(inputs):
    """Host-side prep shared across cores (weights/constants)."""
    f32 = np.float32
    Wq = np.asarray(inputs["Wq"], f32)
    Wk = np.asarray(inputs["Wk"], f32)
    Wv = np.asarray(inputs["Wv"], f32)
    Wo = np.asarray(inputs["Wo"], f32)

    def wlay(Wm):
        return np.ascontiguousarray(Wm.reshape(KC, 128, D).transpose(1, 0, 2))

    rel = np.asarray(inputs["rel_emb"], f32)          # [1024, 64]
    ft = rel.T                                         # [64, 1024]
    ft2 = np.ascontiguousarray(np.concatenate([ft, ft], axis=0))  # [128, 1024]

    common = {
        "wq": wlay(Wq), "wk": wlay(Wk), "wv": wlay(Wv), "wo": wlay(Wo),
        "bq2": np.ascontiguousarray(np.asarray(inputs["bq"], f32).reshape(KC, 128).T),
        "bk2": np.ascontiguousarray(np.asarray(inputs["bk"], f32).reshape(KC, 128).T),
        "bv1": np.asarray(inputs["bv"], f32),
        "bo1": np.asarray(inputs["bo"], f32),
        "ft2": ft2,
        "identbf": np.eye(128, dtype=ml_dtypes.bfloat16),
        "negtriubf": (np.triu(np.ones((128, 128), f32), k=1) * -24000.0).astype(ml_dtypes.bfloat16),
        "identf": np.eye(128, dtype=f32),
        "zerobf": np.full((128, 128), -24000.0, ml_dtypes.bfloat16),  # mask fill for R block-0
    }
    return common


def xlay_pub(xb):   # [L, D] -> xT [512, 1024] -> [128, KC, L]
    xt = xb.T   # [D, L]
    return np.ascontiguousarray(xt.reshape(KC, 128, L).transpose(1, 0, 2))


def kernel(**inputs):
    if "nc" not in _cache:
        _cache["nc"] = build_program()
    nc = _cache["nc"]

    common = _prep_core_inputs(inputs)
    x_q = np.asarray(inputs["x_q"], np.float32)
    x_k = np.asarray(inputs["x_k"], np.float32)
    x_v = np.asarray(inputs["x_v"], np.float32)
    xlay = xlay_pub

    in_maps = []
    for b in range(B):
        m = dict(common)
        m["xqt"] = xlay(x_q[b])
        m["xkt"] = xlay(x_k[b])
        m["xvt"] = xlay(x_v[b])
        in_maps.append(m)

    res = run_bass_kernel_spmd(nc, in_maps, list(range(B)))
    out = np.stack([res.results[b]["out"] for b in range(B)])      # [8, 1024, 512]
    attn = np.stack([res.results[b]["attn"] for b in range(B)])    # [8, 8, 1024, 1024]
    return out, attn
